# revision 6
# baseline (speedup 1.0000x reference)
"""Trainium2 Bass kernel for a 4-layer GCN (nn_GCNNet).

Strategy (8 NeuronCores, SPMD single NEFF):
  - Core c owns the contiguous node range [c*6250, (c+1)*6250) and all edges
    whose dst falls in that range (edge sharding by destination).
  - Node features h live transposed in SBUF as hT [128 d, 6250 nodes] f32.
  - Per GCN layer: every core gathers h[src] rows for its edges from a
    replicated DRAM copy of h (dma_gather, 512B rows), aggregates them into
    m^T per 128-dst-node block with one-hot matmuls accumulating in PSUM
    (the one-hot carries the symmetric-norm coefficient per edge), applies
    the layer weight as a [128x128] @ [128x512] matmul, relu+bias on the
    scalar engine, residual-adds into hT, and publishes its updated node
    shard via AllGather so every core has the full h for the next layer.
  - dma_gather indices are int16, so the gather source is addressed as two
    25000-row halves; host pre-sorts each block's edge list into (lo, hi)
    sublists padded to multiples of 128 (pad slots gather row 0 and carry a
    zero norm coefficient, so they contribute nothing).
  - MLP readout (128->64->32->128) runs on the transposed features, then
    tiles are transposed back via the PE and DMA'd out.

Host-side work is limited to graph preprocessing: sharding/sorting edges,
padding, building index streams, degree counts and the norm coefficients
isq_src[src]*isq_dst[dst] (pure functions of the integer edge lists), plus
the constant sinusoidal position table. All tensor math (embedding lookup,
aggregation, matmuls, activations, residuals, readout) runs on device.
"""

import os
import sys

sys.path.insert(0, "/opt/trn_rl_repo")

import math

import numpy as np

import concourse.bacc as bacc
import concourse.bass as bass
import concourse.mybir as mybir
import concourse.tile as tile
from concourse.bass_utils import run_bass_kernel_spmd

# Problem constants (hardcoded per contest rules).
N_GRAPHS = 25
NODES_PER = 2000
N = N_GRAPHS * NODES_PER          # 50000
E = 800000
D = 128
VOCAB = 30
NLAYERS = 4
NCORES = 8
NPC = N // NCORES                 # 6250 nodes per core
HALF = N // 2                     # int16 gather index limit workaround
NB = (NPC + 127) // 128           # 49 dst blocks / node tiles per core
LAST_ROWS = NPC - 128 * (NB - 1)  # 106 valid rows in the last tile
NSLOT = NB * 128                  # 6272 padded node slots
CHUNK_NB = 4                      # dst blocks per gather chunk (= W-matmul group)

F32 = mybir.dt.float32
BF16 = mybir.dt.bfloat16
I16 = mybir.dt.int16

_cache = {}


def _pos_table():
    pos = (np.arange(NODES_PER, dtype=np.float64) + 1.0)[:, None]
    div = np.exp(np.arange(0, D, 2, dtype=np.float64) * (-math.log(10000.0) / D))
    ang = pos * div
    tab = np.stack([np.sin(ang), np.cos(ang)], axis=-1).reshape(NODES_PER, D)
    return tab.astype(np.float32)


def _wrap16(stream):
    """int16 index stream -> [128, len/16] SBUF layout (16-partition wrap,
    replicated to all 8 gpsimd cores)."""
    v = stream.reshape(-1, 16).T  # [16, cols]
    return np.tile(v, (8, 1)).astype(np.int16)


def _preprocess(labels, src, dst, perms):
    """Shard/sort/pad edges; build per-core device input arrays."""
    src = np.asarray(src).astype(np.int64)
    dst = np.asarray(dst).astype(np.int64)
    labels = np.asarray(labels).astype(np.int64)
    perms = np.asarray(perms).astype(np.int64)

    deg_out = np.bincount(src, minlength=N)
    deg_in = np.bincount(dst, minlength=N)
    isq_src = (np.maximum(deg_out, 1) ** -0.5).astype(np.float32)
    isq_dst = (np.maximum(deg_in, 1) ** -0.5).astype(np.float32)
    se_all = (isq_src[src] * isq_dst[dst]).astype(np.float32)

    core = dst // NPC
    dstloc = dst % NPC
    blk = dstloc >> 7
    dl = (dstloc & 127).astype(np.float32)
    half = (src >= HALF).astype(np.int64)
    gid = (core * NB + blk) * 2 + half
    order = np.argsort(gid, kind="stable")
    s_src, s_se, s_dl, s_gid = src[order], se_all[order], dl[order], gid[order]
    counts = np.bincount(gid, minlength=NCORES * NB * 2).reshape(NCORES, NB, 2)
    starts = np.concatenate([[0], np.cumsum(counts.reshape(-1))])[:-1].reshape(
        NCORES, NB, 2
    )

    # Cross-core max tile count per (block, half) — all cores share one NEFF,
    # so the instruction stream (tile counts) must be identical.
    T = np.ceil(counts.max(axis=0) / 128).astype(np.int64)  # [NB, 2]

    # Tile layout: per chunk of CHUNK_NB blocks: all lo tiles (per block),
    # then all hi tiles (per block).
    chunks = []  # (blocks, tile_base, TL, TH)
    tiles_of_block = [None] * NB
    tbase = 0
    for k0 in range(0, NB, CHUNK_NB):
        blocks = list(range(k0, min(NB, k0 + CHUNK_NB)))
        TL = int(sum(T[b, 0] for b in blocks))
        TH = int(sum(T[b, 1] for b in blocks))
        off = tbase
        for b in blocks:
            tiles_of_block[b] = list(range(off, off + T[b, 0]))
            off += T[b, 0]
        for b in blocks:
            tiles_of_block[b] += list(range(off, off + T[b, 1]))
            off += T[b, 1]
        chunks.append((blocks, tbase, TL, TH))
        tbase = off
    ntiles = tbase
    nslot_e = ntiles * 128

    # slot offset of each (block, half) group within the padded stream
    slot_start = np.zeros((NB, 2), np.int64)
    for b in range(NB):
        slot_start[b, 0] = tiles_of_block[b][0] * 128
        slot_start[b, 1] = tiles_of_block[b][int(T[b, 0])] * 128 if T[b, 1] else 0

    # position-encoding inverse permutation (host = pure index preprocessing)
    pos_idx = np.zeros(N, np.int64)
    ar = np.arange(NODES_PER)
    for g in range(N_GRAPHS):
        pos_idx[g * NODES_PER + perms[g]] = ar

    per_core = []
    for c in range(NCORES):
        idx_s = np.zeros(nslot_e, np.int64)
        se_s = np.zeros(nslot_e, np.float32)
        dl_s = np.zeros(nslot_e, np.float32)
        for b in range(NB):
            for h in range(2):
                n = counts[c, b, h]
                if n == 0:
                    continue
                s0 = starts[c, b, h]
                d0 = slot_start[b, h]
                sl = slice(d0, d0 + n)
                idx_s[sl] = s_src[s0 : s0 + n] - (HALF if h else 0)
                se_s[sl] = s_se[s0 : s0 + n]
                dl_s[sl] = s_dl[s0 : s0 + n]
        lab_s = np.zeros(NSLOT, np.int64)
        lab_s[:NPC] = labels[c * NPC : (c + 1) * NPC]
        pos_s = np.zeros(NSLOT, np.int64)
        pos_s[:NPC] = pos_idx[c * NPC : (c + 1) * NPC]
        idxall = np.concatenate(
            [_wrap16(idx_s), _wrap16(lab_s), _wrap16(pos_s)], axis=1
        )
        per_core.append(
            dict(
                idxall=idxall,
                dl=dl_s.reshape(ntiles, 128).T.copy(),
                se=se_s.reshape(ntiles, 128).T.copy(),
            )
        )

    meta = dict(chunks=chunks, tiles_of_block=tiles_of_block, ntiles=ntiles)
    return meta, per_core


def _build_cst(ntiles, dl, se, Ws, bs, w1, b1, w2, b2, w3, b3):
    """One [128, CSTW] f32 constant block -> single DMA, single dep."""
    cols = {}
    parts = []
    off = 0

    def add(name, arr):
        nonlocal off
        a = np.zeros((128, arr.shape[1]), np.float32)
        a[: arr.shape[0]] = arr
        cols[name] = off
        parts.append(a)
        off += arr.shape[1]

    import ml_dtypes

    def addb(name, arr_bf16):
        # pack a [128, W] bf16 array into W/2 f32 columns (bitcast on device)
        a = np.zeros((128, arr_bf16.shape[1]), ml_dtypes.bfloat16)
        a[: arr_bf16.shape[0]] = arr_bf16
        add(name, a.view(np.float32))

    add("iota", np.tile(np.arange(128, dtype=np.float32), (128, 1)))
    addb("iota_b", np.tile(np.arange(128), (128, 1)).astype(ml_dtypes.bfloat16))

    add("ident", np.eye(128, dtype=np.float32))
    add("dl", dl)
    add("se", se)
    add("W4", np.concatenate([np.asarray(Ws[l], np.float32) for l in range(NLAYERS)], 1))
    add("b4", np.stack([np.asarray(bs[l], np.float32) for l in range(NLAYERS)], 1))
    add("w1", np.asarray(w1, np.float32))
    add("b1", np.asarray(b1, np.float32)[:, None])
    add("w2", np.asarray(w2, np.float32))
    add("b2", np.asarray(b2, np.float32)[:, None])
    add("w3", np.asarray(w3, np.float32))
    add("b3", np.asarray(b3, np.float32)[:, None])
    return np.concatenate(parts, axis=1), cols


def _build_nc(meta, cstw, ccols, idxw):
    chunks = meta["chunks"]
    tiles_of_block = meta["tiles_of_block"]
    ntiles = meta["ntiles"]
    ctmax = max(TL + TH for _, _, TL, TH in chunks)

    nc = bacc.Bacc("TRN2", target_bir_lowering=False, debug=False, num_devices=NCORES)
    idxall = nc.dram_tensor("idxall", [128, idxw], I16, kind="ExternalInput").ap()
    cst_in = nc.dram_tensor("cst", [128, cstw], F32, kind="ExternalInput").ap()
    w4b_in = nc.dram_tensor("w4b", [128, NLAYERS * D], BF16, kind="ExternalInput").ap()
    emb_in = nc.dram_tensor("emb", [VOCAB, D], F32, kind="ExternalInput").ap()
    pos_in = nc.dram_tensor("pos", [NODES_PER, D], F32, kind="ExternalInput").ap()
    out_d = nc.dram_tensor("out", [NPC, D], F32, kind="ExternalOutput").ap()

    is_eq = mybir.AluOpType.is_equal
    mult = mybir.AluOpType.mult
    Relu = mybir.ActivationFunctionType.Relu
    Ident = mybir.ActivationFunctionType.Identity

    with tile.TileContext(nc) as tc:
        with (
            tc.tile_pool(name="persist", bufs=1) as pp,
            tc.tile_pool(name="g", bufs=2) as gpool,
            tc.tile_pool(name="oh", bufs=6) as ohpool,
            tc.tile_pool(name="mt", bufs=2) as mtpool,
            tc.tile_pool(name="zr", bufs=2) as zrpool,
            tc.tile_pool(name="hb", bufs=4) as hbpool,
            tc.tile_pool(name="ro", bufs=2) as ropool,
            tc.tile_pool(name="psm", bufs=2, space="PSUM") as psm,
            tc.tile_pool(name="psz", bufs=2, space="PSUM") as psz,
            tc.tile_pool(name="pst", bufs=2, space="PSUM") as pst,
            tc.tile_pool(name="psr", bufs=2, space="PSUM") as psr,
            tc.tile_pool(name="dram", bufs=1, space="DRAM") as dram,
        ):
            idx_t = pp.tile([128, idxw], I16, tag="idx")
            nc.sync.dma_start(idx_t[:], idxall[:])
            cst = pp.tile([128, cstw], F32, tag="cst")
            nc.sync.dma_start(cst[:], cst_in[:])
            hT = pp.tile([128, NSLOT], F32, tag="hT")

            def cc(name, j=0, rows=128, w=1):
                return cst[0:rows, ccols[name] + j : ccols[name] + j + w]

            iota_ap = cc("iota", w=128)
            iota_b_ap = cc("iota_b", w=64).bitcast(BF16)
            w4b = pp.tile([128, NLAYERS * D], BF16, tag="w4b")
            nc.sync.dma_start(w4b[:], w4b_in[:])
            W4b_ap_all = w4b
            ident_ap = cc("ident", w=128)

            hg = dram.tile([N, D], BF16, tag="hg")
            hgb = dram.tile([NPC, D], BF16, tag="hgb")

            def allgather():
                nc.gpsimd.collective_compute(
                    "AllGather",
                    mybir.AluOpType.bypass,
                    replica_groups=[list(range(NCORES))],
                    ins=[hgb.opt()],
                    outs=[hg.opt()],
                )

            def writeback(src_ap_of_tile):
                for b in range(NB):
                    rows = LAST_ROWS if b == NB - 1 else 128
                    hb = hbpool.tile([128, 128], F32, tag="hb")
                    nc.scalar.copy(hb[:], src_ap_of_tile(b))
                    nc.sync.dma_start(hgb[b * 128 : b * 128 + rows, :], hb[0:rows, :])

            # ---- setup: h0 = emb[labels] + pos_table[inv_perm] ----
            ge = gpool.tile([128, ctmax * 128], F32, tag="g")
            gp = gpool.tile([128, ctmax * 128], F32, tag="g")
            e0 = ntiles * 8
            nc.gpsimd.dma_gather(
                ge[:, 0:NSLOT].rearrange("p (t e) -> p t e", e=D),
                emb_in[:, :],
                idx_t[:, e0 : e0 + NSLOT // 16],
                NSLOT, NSLOT, D, single_packet=False,
            )
            nc.gpsimd.dma_gather(
                gp[:, 0:NSLOT].rearrange("p (t e) -> p t e", e=D),
                pos_in[:, :],
                idx_t[:, e0 + NSLOT // 16 : e0 + 2 * (NSLOT // 16)],
                NSLOT, NSLOT, D, single_packet=False,
            )
            nc.vector.tensor_add(ge[:, 0:NSLOT], ge[:, 0:NSLOT], gp[:, 0:NSLOT])
            # h0 rows -> bf16 -> DRAM bounce (natural layout), + transposed into hT
            hb0 = pp.tile([128, NSLOT], BF16, tag="hb0")
            nc.scalar.copy(hb0[:], ge[:, 0:NSLOT])
            for b in range(NB):
                rows = LAST_ROWS if b == NB - 1 else 128
                nc.sync.dma_start(
                    hgb[b * 128 : b * 128 + rows, :],
                    hb0[0:rows, b * 128 : (b + 1) * 128],
                )
            for b in range(NB):
                pt = pst.tile([128, 128], F32, tag="pt")
                nc.tensor.transpose(pt[:], ge[:, b * 128 : (b + 1) * 128], ident_ap)
                nc.scalar.copy(hT[:, b * 128 : (b + 1) * 128], pt[:])
            allgather()

            # ---- GCN layers ----
            for l in range(int(os.environ.get("GCN_NLAYERS", NLAYERS))):
                for blocks, t0, TL, TH in chunks:
                    g = gpool.tile([128, ctmax * 128], BF16, tag="g")
                    if TL:
                        nc.gpsimd.dma_gather(
                            g[:, 0 : TL * 128].rearrange("p (t e) -> p t e", e=D),
                            hg[0:HALF, :],
                            idx_t[:, t0 * 8 : (t0 + TL) * 8],
                            TL * 128, TL * 128, D, single_packet=False,
                        )
                    if TH:
                        nc.gpsimd.dma_gather(
                            g[:, TL * 128 : (TL + TH) * 128].rearrange(
                                "p (t e) -> p t e", e=D
                            ),
                            hg[HALF:, :],
                            idx_t[:, (t0 + TL) * 8 : (t0 + TL + TH) * 8],
                            TH * 128, TH * 128, D, single_packet=False,
                        )
                    mT = mtpool.tile([128, 512], BF16, tag="mT")
                    for j, b in enumerate(blocks):
                        pm = psm.tile([128, 128], F32, tag="pm")
                        tl = tiles_of_block[b]
                        for i, t in enumerate(tl):
                            oh = ohpool.tile([128, 128], BF16, tag="oh")
                            nc.vector.tensor_scalar(
                                oh[:], iota_b_ap,
                                cc("dl", t), cc("se", t),
                                is_eq, mult,
                            )
                            nc.tensor.matmul(
                                pm[:],
                                g[:, (t - t0) * 128 : (t - t0 + 1) * 128],
                                oh[:],
                                start=(i == 0),
                                stop=(i == len(tl) - 1),
                            )
                        nc.scalar.copy(mT[:, j * 128 : (j + 1) * 128], pm[:])
                    cols = len(blocks) * 128
                    pz = psz.tile([128, 512], F32, tag="pz")
                    nc.tensor.matmul(
                        pz[:, 0:cols],
                        w4b[:, l * 128 : (l + 1) * 128],
                        mT[:, 0:cols],
                        start=True, stop=True,
                    )
                    zr = zrpool.tile([128, 512], F32, tag="zr")
                    nc.scalar.activation(
                        zr[:, 0:cols], pz[:, 0:cols], Relu, bias=cc("b4", l)
                    )
                    c0 = blocks[0] * 128
                    nc.vector.tensor_add(
                        hT[:, c0 : c0 + cols], hT[:, c0 : c0 + cols], zr[:, 0:cols]
                    )
                    if l < NLAYERS - 1:
                        for b in blocks:
                            rows = LAST_ROWS if b == NB - 1 else 128
                            pt = pst.tile([128, 128], F32, tag="pt")
                            nc.tensor.transpose(
                                pt[:], hT[:, b * 128 : (b + 1) * 128], ident_ap
                            )
                            hb = hbpool.tile([128, 128], BF16, tag="hbw")
                            nc.scalar.copy(hb[:], pt[:])
                            nc.sync.dma_start(
                                hgb[b * 128 : b * 128 + rows, :], hb[0:rows, :]
                            )
                if l < NLAYERS - 1:
                    allgather()

            # ---- MLP readout ----
            for off in range(0, NSLOT, 512):
                cols = min(512, NSLOT - off)
                p1 = psr.tile([64, 512], F32, tag="pro")
                nc.tensor.matmul(
                    p1[:, 0:cols], cc("w1", rows=128, w=64), hT[:, off : off + cols],
                    start=True, stop=True,
                )
                x1 = ropool.tile([64, 512], F32, tag="x1")
                nc.scalar.activation(
                    x1[:, 0:cols], p1[:, 0:cols], Relu, bias=cc("b1", rows=64)
                )
                p2 = psr.tile([32, 512], F32, tag="pro")
                nc.tensor.matmul(
                    p2[:, 0:cols], cc("w2", rows=64, w=32), x1[:, 0:cols],
                    start=True, stop=True,
                )
                x2 = ropool.tile([32, 512], F32, tag="x2")
                nc.scalar.activation(
                    x2[:, 0:cols], p2[:, 0:cols], Relu, bias=cc("b2", rows=32)
                )
                p3 = psr.tile([128, 512], F32, tag="pro")
                nc.tensor.matmul(
                    p3[:, 0:cols], cc("w3", rows=32, w=128), x2[:, 0:cols],
                    start=True, stop=True,
                )
                x3 = ropool.tile([128, 512], F32, tag="x3")
                nc.scalar.activation(
                    x3[:, 0:cols], p3[:, 0:cols], Ident, bias=cc("b3")
                )
                for j in range(0, cols, 128):
                    b = (off + j) // 128
                    rows = LAST_ROWS if b == NB - 1 else 128
                    pt = pst.tile([128, 128], F32, tag="pt")
                    nc.tensor.transpose(pt[:], x3[:, j : j + 128], ident_ap)
                    ob = hbpool.tile([128, 128], F32, tag="hb")
                    nc.scalar.copy(ob[:], pt[:])
                    nc.sync.dma_start(
                        out_d[b * 128 : b * 128 + rows, :], ob[0:rows, :]
                    )
    nc.compile()
    return nc


last_results = None


def kernel(labels, src, dst, perms, emb, Ws, bs, w1, b1, w2, b2, w3, b3):
    global last_results
    meta, per_core = _preprocess(labels, src, dst, perms)
    cst0, ccols = _build_cst(
        meta["ntiles"], per_core[0]["dl"], per_core[0]["se"],
        Ws, bs, w1, b1, w2, b2, w3, b3,
    )
    key = (meta["ntiles"], os.environ.get("GCN_NLAYERS", ""), tuple(t for _, t, _, _ in meta["chunks"]))
    if key not in _cache:
        _cache[key] = _build_nc(
            meta, cst0.shape[1], ccols, per_core[0]["idxall"].shape[1]
        )
    nc = _cache[key]

    emb_np = np.asarray(emb, np.float32)
    pos_np = _pos_table()
    import ml_dtypes
    w4b_np = np.concatenate(
        [np.asarray(Ws[l], np.float32) for l in range(NLAYERS)], 1
    ).astype(ml_dtypes.bfloat16)
    in_maps = []
    for c in range(NCORES):
        cst_c, _ = _build_cst(
            meta["ntiles"], per_core[c]["dl"], per_core[c]["se"],
            Ws, bs, w1, b1, w2, b2, w3, b3,
        )
        in_maps.append(
            dict(idxall=per_core[c]["idxall"], cst=cst_c, emb=emb_np, pos=pos_np,
                 w4b=w4b_np)
        )
    res = run_bass_kernel_spmd(nc, in_maps, core_ids=list(range(NCORES)))
    last_results = res
    return np.concatenate([res.results[c]["out"] for c in range(NCORES)], axis=0)


# revision 7
# speedup vs baseline: 1.0561x; 1.0561x over previous
"""Trainium2 Bass kernel for a 4-layer GCN (nn_GCNNet).

Strategy (8 NeuronCores, SPMD single NEFF):
  - Core c owns the contiguous node range [c*6250, (c+1)*6250) and all edges
    whose dst falls in that range (edge sharding by destination).
  - Node features h live transposed in SBUF as hT [128 d, 6250 nodes] f32.
  - Per GCN layer: every core gathers h[src] rows for its edges from a
    replicated DRAM copy of h (dma_gather, 512B rows), aggregates them into
    m^T per 128-dst-node block with one-hot matmuls accumulating in PSUM
    (the one-hot carries the symmetric-norm coefficient per edge), applies
    the layer weight as a [128x128] @ [128x512] matmul, relu+bias on the
    scalar engine, residual-adds into hT, and publishes its updated node
    shard via AllGather so every core has the full h for the next layer.
  - dma_gather indices are int16, so the gather source is addressed as two
    25000-row halves; host pre-sorts each block's edge list into (lo, hi)
    sublists padded to multiples of 128 (pad slots gather row 0 and carry a
    zero norm coefficient, so they contribute nothing).
  - MLP readout (128->64->32->128) runs on the transposed features, then
    tiles are transposed back via the PE and DMA'd out.

Host-side work is limited to graph preprocessing: sharding/sorting edges,
padding, building index streams, degree counts and the norm coefficients
isq_src[src]*isq_dst[dst] (pure functions of the integer edge lists), plus
the constant sinusoidal position table. All tensor math (embedding lookup,
aggregation, matmuls, activations, residuals, readout) runs on device.
"""

import os
import sys

sys.path.insert(0, "/opt/trn_rl_repo")

import math

import numpy as np

import concourse.bacc as bacc
import concourse.bass as bass
import concourse.mybir as mybir
import concourse.tile as tile
from concourse.bass_utils import run_bass_kernel_spmd

# Problem constants (hardcoded per contest rules).
N_GRAPHS = 25
NODES_PER = 2000
N = N_GRAPHS * NODES_PER          # 50000
E = 800000
D = 128
VOCAB = 30
NLAYERS = 4
NCORES = 8
NPC = N // NCORES                 # 6250 nodes per core
HALF = N // 2                     # int16 gather index limit workaround
NB = (NPC + 127) // 128           # 49 dst blocks / node tiles per core
LAST_ROWS = NPC - 128 * (NB - 1)  # 106 valid rows in the last tile
NSLOT = NB * 128                  # 6272 padded node slots
CHUNK_NB = 4                      # dst blocks per gather chunk (= W-matmul group)

F32 = mybir.dt.float32
BF16 = mybir.dt.bfloat16
I16 = mybir.dt.int16

_cache = {}


def _pos_table():
    pos = (np.arange(NODES_PER, dtype=np.float64) + 1.0)[:, None]
    div = np.exp(np.arange(0, D, 2, dtype=np.float64) * (-math.log(10000.0) / D))
    ang = pos * div
    tab = np.stack([np.sin(ang), np.cos(ang)], axis=-1).reshape(NODES_PER, D)
    return tab.astype(np.float32)


def _wrap16(stream):
    """int16 index stream -> [128, len/16] SBUF layout (16-partition wrap,
    replicated to all 8 gpsimd cores)."""
    v = stream.reshape(-1, 16).T  # [16, cols]
    return np.tile(v, (8, 1)).astype(np.int16)


def _preprocess(labels, src, dst, perms):
    """Shard/sort/pad edges; build per-core device input arrays."""
    src = np.asarray(src).astype(np.int64)
    dst = np.asarray(dst).astype(np.int64)
    labels = np.asarray(labels).astype(np.int64)
    perms = np.asarray(perms).astype(np.int64)

    deg_out = np.bincount(src, minlength=N)
    deg_in = np.bincount(dst, minlength=N)
    isq_src = (np.maximum(deg_out, 1) ** -0.5).astype(np.float32)
    isq_dst = (np.maximum(deg_in, 1) ** -0.5).astype(np.float32)
    se_all = (isq_src[src] * isq_dst[dst]).astype(np.float32)

    core = dst // NPC
    dstloc = dst % NPC
    blk = dstloc >> 7
    dl = (dstloc & 127).astype(np.float32)
    half = (src >= HALF).astype(np.int64)
    gid = (core * NB + blk) * 2 + half
    order = np.argsort(gid, kind="stable")
    s_src, s_se, s_dl, s_gid = src[order], se_all[order], dl[order], gid[order]
    counts = np.bincount(gid, minlength=NCORES * NB * 2).reshape(NCORES, NB, 2)
    starts = np.concatenate([[0], np.cumsum(counts.reshape(-1))])[:-1].reshape(
        NCORES, NB, 2
    )

    # Cross-core max tile count per (block, half) — all cores share one NEFF,
    # so the instruction stream (tile counts) must be identical.
    T = np.ceil(counts.max(axis=0) / 128).astype(np.int64)  # [NB, 2]

    # Tile layout: per chunk of CHUNK_NB blocks: all lo tiles (per block),
    # then all hi tiles (per block).
    chunks = []  # (blocks, tile_base, TL, TH)
    tiles_of_block = [None] * NB
    tbase = 0
    for k0 in range(0, NB, CHUNK_NB):
        blocks = list(range(k0, min(NB, k0 + CHUNK_NB)))
        TL = int(sum(T[b, 0] for b in blocks))
        TH = int(sum(T[b, 1] for b in blocks))
        off = tbase
        for b in blocks:
            tiles_of_block[b] = list(range(off, off + T[b, 0]))
            off += T[b, 0]
        for b in blocks:
            tiles_of_block[b] += list(range(off, off + T[b, 1]))
            off += T[b, 1]
        chunks.append((blocks, tbase, TL, TH))
        tbase = off
    ntiles = tbase
    nslot_e = ntiles * 128

    # slot offset of each (block, half) group within the padded stream
    slot_start = np.zeros((NB, 2), np.int64)
    for b in range(NB):
        slot_start[b, 0] = tiles_of_block[b][0] * 128
        slot_start[b, 1] = tiles_of_block[b][int(T[b, 0])] * 128 if T[b, 1] else 0

    # position-encoding inverse permutation (host = pure index preprocessing)
    pos_idx = np.zeros(N, np.int64)
    ar = np.arange(NODES_PER)
    for g in range(N_GRAPHS):
        pos_idx[g * NODES_PER + perms[g]] = ar

    per_core = []
    for c in range(NCORES):
        idx_s = np.zeros(nslot_e, np.int64)
        se_s = np.zeros(nslot_e, np.float32)
        dl_s = np.zeros(nslot_e, np.float32)
        for b in range(NB):
            for h in range(2):
                n = counts[c, b, h]
                if n == 0:
                    continue
                s0 = starts[c, b, h]
                d0 = slot_start[b, h]
                sl = slice(d0, d0 + n)
                idx_s[sl] = s_src[s0 : s0 + n] - (HALF if h else 0)
                se_s[sl] = s_se[s0 : s0 + n]
                dl_s[sl] = s_dl[s0 : s0 + n]
        lab_s = np.zeros(NSLOT, np.int64)
        lab_s[:NPC] = labels[c * NPC : (c + 1) * NPC]
        pos_s = np.zeros(NSLOT, np.int64)
        pos_s[:NPC] = pos_idx[c * NPC : (c + 1) * NPC]
        idxall = np.concatenate(
            [_wrap16(idx_s), _wrap16(lab_s), _wrap16(pos_s)], axis=1
        )
        per_core.append(
            dict(
                idxall=idxall,
                dl=dl_s.reshape(ntiles, 128).T.copy(),
                se=se_s.reshape(ntiles, 128).T.copy(),
            )
        )

    meta = dict(chunks=chunks, tiles_of_block=tiles_of_block, ntiles=ntiles)
    return meta, per_core


def _build_cst(ntiles, dl, se, Ws, bs, w1, b1, w2, b2, w3, b3):
    """One [128, CSTW] f32 constant block -> single DMA, single dep."""
    cols = {}
    parts = []
    off = 0

    def add(name, arr):
        nonlocal off
        a = np.zeros((128, arr.shape[1]), np.float32)
        a[: arr.shape[0]] = arr
        cols[name] = off
        parts.append(a)
        off += arr.shape[1]

    import ml_dtypes

    def addb(name, arr_bf16):
        # pack a [128, W] bf16 array into W/2 f32 columns (bitcast on device)
        a = np.zeros((128, arr_bf16.shape[1]), ml_dtypes.bfloat16)
        a[: arr_bf16.shape[0]] = arr_bf16
        add(name, a.view(np.float32))

    add("iota", np.tile(np.arange(128, dtype=np.float32), (128, 1)))
    addb("iota_b", np.tile(np.arange(128), (128, 1)).astype(ml_dtypes.bfloat16))

    add("ident", np.eye(128, dtype=np.float32))
    add("dl", dl)
    add("se", se)
    add("dln", -dl)
    add("sen", -se)
    add("W4", np.concatenate([np.asarray(Ws[l], np.float32) for l in range(NLAYERS)], 1))
    add("b4", np.stack([np.asarray(bs[l], np.float32) for l in range(NLAYERS)], 1))
    add("w1", np.asarray(w1, np.float32))
    add("b1", np.asarray(b1, np.float32)[:, None])
    add("w2", np.asarray(w2, np.float32))
    add("b2", np.asarray(b2, np.float32)[:, None])
    add("w3", np.asarray(w3, np.float32))
    add("b3", np.asarray(b3, np.float32)[:, None])
    return np.concatenate(parts, axis=1), cols


def _build_nc(meta, cstw, ccols, idxw):
    chunks = meta["chunks"]
    tiles_of_block = meta["tiles_of_block"]
    ntiles = meta["ntiles"]
    ctmax = max(TL + TH for _, _, TL, TH in chunks)

    nc = bacc.Bacc("TRN2", target_bir_lowering=False, debug=False, num_devices=NCORES)
    idxall = nc.dram_tensor("idxall", [128, idxw], I16, kind="ExternalInput").ap()
    cst_in = nc.dram_tensor("cst", [128, cstw], F32, kind="ExternalInput").ap()
    w4b_in = nc.dram_tensor("w4b", [128, NLAYERS * D], BF16, kind="ExternalInput").ap()
    emb_in = nc.dram_tensor("emb", [VOCAB, D], F32, kind="ExternalInput").ap()
    pos_in = nc.dram_tensor("pos", [NODES_PER, D], F32, kind="ExternalInput").ap()
    out_d = nc.dram_tensor("out", [NPC, D], F32, kind="ExternalOutput").ap()

    is_eq = mybir.AluOpType.is_equal
    mult = mybir.AluOpType.mult
    Relu = mybir.ActivationFunctionType.Relu
    Square = mybir.ActivationFunctionType.Square
    Ident = mybir.ActivationFunctionType.Identity
    ONEHOT_ENGINE = os.environ.get("GCN_ONEHOT", "act")

    with tile.TileContext(nc) as tc:
        with (
            tc.tile_pool(name="persist", bufs=1) as pp,
            tc.tile_pool(name="g", bufs=2) as gpool,
            tc.tile_pool(name="oh", bufs=6) as ohpool,
            tc.tile_pool(name="mt", bufs=2) as mtpool,
            tc.tile_pool(name="zr", bufs=2) as zrpool,
            tc.tile_pool(name="hb", bufs=4) as hbpool,
            tc.tile_pool(name="ro", bufs=2) as ropool,
            tc.tile_pool(name="psm", bufs=2, space="PSUM") as psm,
            tc.tile_pool(name="psz", bufs=2, space="PSUM") as psz,
            tc.tile_pool(name="pst", bufs=2, space="PSUM") as pst,
            tc.tile_pool(name="psr", bufs=2, space="PSUM") as psr,
            tc.tile_pool(name="dram", bufs=1, space="DRAM") as dram,
        ):
            idx_t = pp.tile([128, idxw], I16, tag="idx")
            nc.sync.dma_start(idx_t[:], idxall[:])
            cst = pp.tile([128, cstw], F32, tag="cst")
            nc.sync.dma_start(cst[:], cst_in[:])
            hT = pp.tile([128, NSLOT], F32, tag="hT")

            def cc(name, j=0, rows=128, w=1):
                return cst[0:rows, ccols[name] + j : ccols[name] + j + w]

            iota_ap = cc("iota", w=128)
            iota_b_ap = cc("iota_b", w=64).bitcast(BF16)
            w4b = pp.tile([128, NLAYERS * D], BF16, tag="w4b")
            nc.sync.dma_start(w4b[:], w4b_in[:])
            W4b_ap_all = w4b
            ident_ap = cc("ident", w=128)

            hg = dram.tile([N, D], BF16, tag="hg")
            hgb = dram.tile([NPC, D], BF16, tag="hgb")

            def allgather():
                nc.gpsimd.collective_compute(
                    "AllGather",
                    mybir.AluOpType.bypass,
                    replica_groups=[list(range(NCORES))],
                    ins=[hgb.opt()],
                    outs=[hg.opt()],
                )

            def writeback(src_ap_of_tile):
                for b in range(NB):
                    rows = LAST_ROWS if b == NB - 1 else 128
                    hb = hbpool.tile([128, 128], F32, tag="hb")
                    nc.scalar.copy(hb[:], src_ap_of_tile(b))
                    nc.sync.dma_start(hgb[b * 128 : b * 128 + rows, :], hb[0:rows, :])

            # ---- setup: h0 = emb[labels] + pos_table[inv_perm] ----
            ge = gpool.tile([128, ctmax * 128], F32, tag="g")
            gp = gpool.tile([128, ctmax * 128], F32, tag="g")
            e0 = ntiles * 8
            nc.gpsimd.dma_gather(
                ge[:, 0:NSLOT].rearrange("p (t e) -> p t e", e=D),
                emb_in[:, :],
                idx_t[:, e0 : e0 + NSLOT // 16],
                NSLOT, NSLOT, D, single_packet=False,
            )
            nc.gpsimd.dma_gather(
                gp[:, 0:NSLOT].rearrange("p (t e) -> p t e", e=D),
                pos_in[:, :],
                idx_t[:, e0 + NSLOT // 16 : e0 + 2 * (NSLOT // 16)],
                NSLOT, NSLOT, D, single_packet=False,
            )
            nc.vector.tensor_add(ge[:, 0:NSLOT], ge[:, 0:NSLOT], gp[:, 0:NSLOT])
            # h0 rows -> bf16 -> DRAM bounce (natural layout), + transposed into hT
            hb0 = pp.tile([128, NSLOT], BF16, tag="hb0")
            nc.scalar.copy(hb0[:], ge[:, 0:NSLOT])
            for b in range(NB):
                rows = LAST_ROWS if b == NB - 1 else 128
                nc.sync.dma_start(
                    hgb[b * 128 : b * 128 + rows, :],
                    hb0[0:rows, b * 128 : (b + 1) * 128],
                )
            for b in range(NB):
                pt = pst.tile([128, 128], F32, tag="pt")
                nc.tensor.transpose(pt[:], ge[:, b * 128 : (b + 1) * 128], ident_ap)
                nc.scalar.copy(hT[:, b * 128 : (b + 1) * 128], pt[:])
            allgather()

            # ---- GCN layers ----
            for l in range(int(os.environ.get("GCN_NLAYERS", NLAYERS))):
                for blocks, t0, TL, TH in chunks:
                    g = gpool.tile([128, ctmax * 128], BF16, tag="g")
                    if TL:
                        nc.gpsimd.dma_gather(
                            g[:, 0 : TL * 128].rearrange("p (t e) -> p t e", e=D),
                            hg[0:HALF, :],
                            idx_t[:, t0 * 8 : (t0 + TL) * 8],
                            TL * 128, TL * 128, D, single_packet=False,
                        )
                    if TH:
                        nc.gpsimd.dma_gather(
                            g[:, TL * 128 : (TL + TH) * 128].rearrange(
                                "p (t e) -> p t e", e=D
                            ),
                            hg[HALF:, :],
                            idx_t[:, (t0 + TL) * 8 : (t0 + TL + TH) * 8],
                            TH * 128, TH * 128, D, single_packet=False,
                        )
                    mT = mtpool.tile([128, 512], BF16, tag="mT")
                    for j, b in enumerate(blocks):
                        pm = psm.tile([128, 128], F32, tag="pm")
                        tl = tiles_of_block[b]
                        for i, t in enumerate(tl):
                            oh = ohpool.tile([128, 128], BF16, tag="oh")
                            if ONEHOT_ENGINE == "dve":
                                nc.vector.tensor_scalar(
                                    oh[:], iota_b_ap,
                                    cc("dl", t), cc("se", t),
                                    is_eq, mult,
                                )
                            else:
                                y = ohpool.tile([128, 128], BF16, tag="ohy")
                                nc.scalar.activation(
                                    y[:], iota_b_ap, Square, bias=cc("dln", t)
                                )
                                nc.scalar.activation(
                                    oh[:], y[:], Relu,
                                    bias=cc("se", t), scale=cc("sen", t),
                                )
                            nc.tensor.matmul(
                                pm[:],
                                g[:, (t - t0) * 128 : (t - t0 + 1) * 128],
                                oh[:],
                                start=(i == 0),
                                stop=(i == len(tl) - 1),
                            )
                        nc.scalar.copy(mT[:, j * 128 : (j + 1) * 128], pm[:])
                    cols = len(blocks) * 128
                    pz = psz.tile([128, 512], F32, tag="pz")
                    nc.tensor.matmul(
                        pz[:, 0:cols],
                        w4b[:, l * 128 : (l + 1) * 128],
                        mT[:, 0:cols],
                        start=True, stop=True,
                    )
                    zr = zrpool.tile([128, 512], F32, tag="zr")
                    nc.scalar.activation(
                        zr[:, 0:cols], pz[:, 0:cols], Relu, bias=cc("b4", l)
                    )
                    c0 = blocks[0] * 128
                    nc.vector.tensor_add(
                        hT[:, c0 : c0 + cols], hT[:, c0 : c0 + cols], zr[:, 0:cols]
                    )
                    if l < NLAYERS - 1:
                        for b in blocks:
                            rows = LAST_ROWS if b == NB - 1 else 128
                            pt = pst.tile([128, 128], F32, tag="pt")
                            nc.tensor.transpose(
                                pt[:], hT[:, b * 128 : (b + 1) * 128], ident_ap
                            )
                            hb = hbpool.tile([128, 128], BF16, tag="hbw")
                            nc.scalar.copy(hb[:], pt[:])
                            nc.sync.dma_start(
                                hgb[b * 128 : b * 128 + rows, :], hb[0:rows, :]
                            )
                if l < NLAYERS - 1:
                    allgather()

            # ---- MLP readout ----
            for off in range(0, NSLOT, 512):
                cols = min(512, NSLOT - off)
                p1 = psr.tile([64, 512], F32, tag="pro")
                nc.tensor.matmul(
                    p1[:, 0:cols], cc("w1", rows=128, w=64), hT[:, off : off + cols],
                    start=True, stop=True,
                )
                x1 = ropool.tile([64, 512], F32, tag="x1")
                nc.scalar.activation(
                    x1[:, 0:cols], p1[:, 0:cols], Relu, bias=cc("b1", rows=64)
                )
                p2 = psr.tile([32, 512], F32, tag="pro")
                nc.tensor.matmul(
                    p2[:, 0:cols], cc("w2", rows=64, w=32), x1[:, 0:cols],
                    start=True, stop=True,
                )
                x2 = ropool.tile([32, 512], F32, tag="x2")
                nc.scalar.activation(
                    x2[:, 0:cols], p2[:, 0:cols], Relu, bias=cc("b2", rows=32)
                )
                p3 = psr.tile([128, 512], F32, tag="pro")
                nc.tensor.matmul(
                    p3[:, 0:cols], cc("w3", rows=32, w=128), x2[:, 0:cols],
                    start=True, stop=True,
                )
                x3 = ropool.tile([128, 512], F32, tag="x3")
                nc.scalar.activation(
                    x3[:, 0:cols], p3[:, 0:cols], Ident, bias=cc("b3")
                )
                for j in range(0, cols, 128):
                    b = (off + j) // 128
                    rows = LAST_ROWS if b == NB - 1 else 128
                    pt = pst.tile([128, 128], F32, tag="pt")
                    nc.tensor.transpose(pt[:], x3[:, j : j + 128], ident_ap)
                    ob = hbpool.tile([128, 128], F32, tag="hb")
                    nc.scalar.copy(ob[:], pt[:])
                    nc.sync.dma_start(
                        out_d[b * 128 : b * 128 + rows, :], ob[0:rows, :]
                    )
    nc.compile()
    return nc


last_results = None


def kernel(labels, src, dst, perms, emb, Ws, bs, w1, b1, w2, b2, w3, b3):
    global last_results
    meta, per_core = _preprocess(labels, src, dst, perms)
    cst0, ccols = _build_cst(
        meta["ntiles"], per_core[0]["dl"], per_core[0]["se"],
        Ws, bs, w1, b1, w2, b2, w3, b3,
    )
    key = (meta["ntiles"], os.environ.get("GCN_NLAYERS", ""), os.environ.get("GCN_ONEHOT", "act"), tuple(t for _, t, _, _ in meta["chunks"]))
    if key not in _cache:
        _cache[key] = _build_nc(
            meta, cst0.shape[1], ccols, per_core[0]["idxall"].shape[1]
        )
    nc = _cache[key]

    emb_np = np.asarray(emb, np.float32)
    pos_np = _pos_table()
    import ml_dtypes
    w4b_np = np.concatenate(
        [np.asarray(Ws[l], np.float32) for l in range(NLAYERS)], 1
    ).astype(ml_dtypes.bfloat16)
    in_maps = []
    for c in range(NCORES):
        cst_c, _ = _build_cst(
            meta["ntiles"], per_core[c]["dl"], per_core[c]["se"],
            Ws, bs, w1, b1, w2, b2, w3, b3,
        )
        in_maps.append(
            dict(idxall=per_core[c]["idxall"], cst=cst_c, emb=emb_np, pos=pos_np,
                 w4b=w4b_np)
        )
    res = run_bass_kernel_spmd(nc, in_maps, core_ids=list(range(NCORES)))
    last_results = res
    return np.concatenate([res.results[c]["out"] for c in range(NCORES)], axis=0)
